# revision 25
# baseline (speedup 1.0000x reference)
"""3-layer GCN + img@pair_embed.T for Trainium2, distributed over 8 NeuronCores.

Strategy (destination-sharded graph parallelism, agg1-exchange variant):
  - Each core owns a contiguous slab of destination nodes (3567, padded 3584).
  - Edges (plus self-loops) are bucketed per 256-destination tile and padded to
    128-edge chunks. Host builds per chunk a dense [128 edges x 256 dests]
    one-hot norm matrix S, so segment-sum aggregation becomes TensorE matmuls.
  - Layer-1 source rows are PRE-GATHERED ON HOST (X is a static input), and the
    layer-1 aggregation computes agg1 = A@X directly in node-row orientation
    (lhsT = S chunk), so agg1 [SLAB, 512] is written without any transpose.
  - KEY: the cross-core exchange moves agg1 (512 wide) instead of h1 (2048
    wide): ONE AllGather of [SLAB,512] -> [8*SLAB,512] (29MB out) instead of
    117MB. Each core then recomputes h1 = relu(agg1 @ W1) for only the unique
    source rows its layer-2/3 edges touch (~13k rows): gather agg1 rows,
    PE-transpose them into contraction layout, GEMM against resident W1.
  - Layer 2 gathers 1024-wide half-rows of the local recomputed h1_u in two
    passes (PSUM has only 8 accumulation banks), GEMMs in dtile pairs
    (free dim 512), and folds img into layer 3: W3img = W3@img.T, Q = h2@W3img.
  - Layer 3 aggregates 64-wide Q after a small Q AllGather.
  - Everything exchanged/gathered travels bf16; W1 float32r; W2/W3img bf16;
    PSUM accumulation fp32.
"""

import numpy as np

from concourse import bacc, bass, mybir
from concourse import tile as tile_mod
from concourse.bass_utils import run_bass_kernel_spmd

# Problem shapes (hardcoded per spec nn_GraphModel_26268019982828)
N = 28535
E = 113000
D = 512
H = 2048
B = 64
N_SKIP = 115 + 245  # attrs + objs; pair nodes are N_SKIP..N-1

NCORES = 8
NODES_PER = -(-N // NCORES)  # 3567
P = 128
DT = 256  # destination tile width
NDT = 14  # dest tiles per core
SLAB = NDT * DT  # 3584 padded dests per core
NFI1 = D // P  # 4 feature chunks of layer-1 width
NFI2 = H // P  # 16 feature chunks of hidden width

f32 = mybir.dt.float32
f32r = mybir.dt.float32r
bf16 = mybir.dt.bfloat16
i32 = mybir.dt.int32


def _round_fp32r(x: np.ndarray) -> np.ndarray:
    """Round-to-nearest-even fp32 -> fp32r (11-bit mantissa), numpy."""
    u = np.ascontiguousarray(x, dtype=np.float32).view(np.uint32)
    r = u + (0x7FF + ((u >> 12) & np.uint32(1)))
    r &= np.uint32(0xFFFFF000)
    return r.view(np.float32)


def _preprocess(edge_index: np.ndarray):
    """Sort/bucket edges by destination; build gather indices + S blocks."""
    src = np.concatenate([edge_index[0], np.arange(N, dtype=np.int64)])
    dst = np.concatenate([edge_index[1], np.arange(N, dtype=np.int64)])
    deg = np.bincount(dst, minlength=N).astype(np.float32)  # includes loops
    dinv = (1.0 / np.sqrt(deg)).astype(np.float32)
    norm = (dinv[src] * dinv[dst]).astype(np.float32)

    core = (dst // NODES_PER).astype(np.int64)
    local = (dst - core * NODES_PER).astype(np.int64)
    t_idx = local // DT
    d_local = local % DT
    bucket = core * NDT + t_idx

    # secondary key: source's q-quarter so layer-3 chunks gate on the
    # earliest quarter AllGather that covers all their sources
    quart = (src % NODES_PER) // (SLAB // 4)
    order = np.argsort(bucket * 4 + quart, kind="stable")
    src_s = src[order]
    bucket_s = bucket[order]
    dl_s = d_local[order]
    norm_s = norm[order]

    counts = np.bincount(bucket_s, minlength=NCORES * NDT)
    ECH = int(-(-counts.max() // P))

    idxA = np.zeros((NCORES, NDT, P, ECH), dtype=np.int32)
    idxB = np.zeros((NCORES, NDT, P, ECH), dtype=np.int32)
    S = np.zeros((NCORES, NDT, P, ECH * DT), dtype=np.float32)

    starts = np.zeros(NCORES * NDT + 1, dtype=np.int64)
    np.cumsum(counts, out=starts[1:])
    pos = np.arange(len(bucket_s)) - starts[bucket_s]
    c_idx = pos // P
    e_idx = pos % P

    ci = bucket_s // NDT
    ti = bucket_s % NDT
    srcB = (src_s // NODES_PER) * SLAB + (src_s % NODES_PER)
    idxA[ci, ti, e_idx, c_idx] = src_s.astype(np.int32)
    idxB[ci, ti, e_idx, c_idx] = srcB.astype(np.int32)
    S[ci, ti, e_idx, c_idx * DT + dl_s] = norm_s
    # gate[t][c] = max source-quarter of chunk c across cores (pads -> 0)
    quart_s = quart[order]
    cnt = counts.reshape(NCORES, NDT)
    qmax = np.zeros((NCORES, NDT, ECH), dtype=np.int64)
    for k in range(NCORES):
        for t in range(NDT):
            b = k * NDT + t
            qs = quart_s[starts[b] : starts[b] + cnt[k, t]]
            for c in range(ECH):
                last = min((c + 1) * P, cnt[k, t]) - 1
                qmax[k, t, c] = qs[last] if last >= c * P else 0
    gate = tuple(
        tuple(int(qmax[:, t, c].max()) for c in range(ECH)) for t in range(NDT)
    )
    return ECH, idxA, idxB, S, gate


def _build(ECH: int, NUCH: int, gateQ, use_b1: bool, use_b2: bool):
    nc = bacc.Bacc("TRN2", target_bir_lowering=False, num_devices=NCORES)
    NU = NUCH * P  # padded unique-source rows per core

    g1_t = nc.dram_tensor("G1", [NDT, P, ECH * D], bf16, kind="ExternalInput")
    w1_t = nc.dram_tensor("W1", [D, H], bf16, kind="ExternalInput")
    w2_t = nc.dram_tensor("W2", [H, H], bf16, kind="ExternalInput")
    w3i_t = nc.dram_tensor("W3img", [H, B], bf16, kind="ExternalInput")
    b1_t = nc.dram_tensor("b1", [1, H], bf16, kind="ExternalInput")
    b2_t = nc.dram_tensor("b2", [P, NFI2], f32r, kind="ExternalInput")
    idxU_t = nc.dram_tensor("idxUT", [P, NUCH], i32, kind="ExternalInput")
    idxL2_t = nc.dram_tensor("idxL2", [NDT, P, 2 * ECH], i32, kind="ExternalInput")
    idxQ2_t = nc.dram_tensor("idxQ2", [P, NDT * ECH], i32, kind="ExternalInput")
    s_tab = nc.dram_tensor("S", [NDT, P, ECH * DT], bf16, kind="ExternalInput")
    ident_t = nc.dram_tensor("IDENT", [P, P], bf16, kind="ExternalInput")
    out_t = nc.dram_tensor("out", [B, SLAB], f32, kind="ExternalOutput")

    agg1p = nc.dram_tensor("agg1p", [SLAB, D], bf16)
    agg1f = nc.dram_tensor("agg1f", [SLAB * NCORES, D], bf16, addr_space="Shared")
    h1u = nc.dram_tensor("h1u", [NU, H], bf16)
    q_slab = nc.dram_tensor("q_slab", [SLAB, B], bf16)
    q_full = nc.dram_tensor("q_full", [SLAB * NCORES, B], bf16, addr_space="Shared")

    rg = [list(range(NCORES))]

    with tile_mod.TileContext(nc) as tc:
        with (
            tc.tile_pool(name="w", bufs=1) as wp,
            tc.tile_pool(name="gio", bufs=6) as gp,
            tc.tile_pool(name="g1io", bufs=2) as g1p,
            tc.tile_pool(name="stab", bufs=2) as sp,
            tc.tile_pool(name="agg", bufs=1) as ap,
            tc.tile_pool(name="small", bufs=3) as mp,
            tc.tile_pool(name="hout", bufs=2) as hp,
            tc.tile_pool(name="consts", bufs=1) as cp,
            tc.tile_pool(name="ps", bufs=8, space="PSUM") as ps,
        ):
            # --- resident constants (W1 loads deferred past layer 1) ---
            idxUall = cp.tile([P, NUCH], i32, tag="idxUall")
            nc.sync.dma_start(out=idxUall[:], in_=idxU_t[:])
            idt = cp.tile([P, P], bf16, tag="idt")
            nc.sync.dma_start(out=idt[:], in_=ident_t[:])
            if use_b1:
                b1sb = cp.tile([1, H], bf16, tag="b1")
                nc.sync.dma_start(out=b1sb[:], in_=b1_t[:])
                ones1 = cp.tile([1, P], bf16, tag="ones")
                nc.gpsimd.memset(ones1[:], 1.0)
            if use_b2:
                b2sb = cp.tile([P, NFI2], f32r, tag="b2")
                nc.sync.dma_start(out=b2sb[:], in_=b2_t[:])

            relu = mybir.ActivationFunctionType.Relu

            # ---------------- Layer 1: agg1 = A@X  (node-row orientation) ---
            for tp in range(NDT // 2):
                g1s, sts = [], []
                for t2 in range(2):
                    t = tp * 2 + t2
                    g1 = g1p.tile([P, ECH * D], bf16, tag="g1")
                    nc.sync.dma_start(out=g1[:], in_=g1_t[t])
                    s_t = sp.tile([P, ECH * DT], bf16, tag="s")
                    nc.sync.dma_start(out=s_t[:], in_=s_tab[t])
                    g1s.append(g1)
                    sts.append(s_t)
                pd = [ps.tile([P, D], f32, tag="ps", name="pd") for _ in range(4)]
                for t2 in range(2):
                    for c in range(ECH):
                        for dh in range(2):
                            nc.tensor.matmul(
                                out=pd[t2 * 2 + dh][:],
                                lhsT=sts[t2][:, c * DT + dh * P : c * DT + (dh + 1) * P],
                                rhs=g1s[t2][:, c * D : (c + 1) * D],
                                start=(c == 0),
                                stop=(c == ECH - 1),
                            )
                for t2 in range(2):
                    for dh in range(2):
                        a1 = mp.tile([P, D], bf16, tag="a1")
                        if dh % 2 == 0:
                            nc.vector.tensor_copy(out=a1[:], in_=pd[t2 * 2 + dh][:])
                        else:
                            nc.scalar.activation(
                                out=a1[:], in_=pd[t2 * 2 + dh][:],
                                func=mybir.ActivationFunctionType.Copy,
                            )
                        nc.sync.dma_start(
                            out=agg1p[
                                (tp * 2 + t2) * DT + dh * P : (tp * 2 + t2) * DT + (dh + 1) * P, :
                            ],
                            in_=a1[:],
                        )

            NOWN = SLAB // P  # 28 chunks of own-slab rows, recomputed locally

            def rec_front(u, src_tab):
                """Gather + transpose chunk u; returns aT tiles."""
                gu = gp.tile([P, D], bf16, tag="gu", bufs=4)
                if u < NOWN:
                    nc.sync.dma_start(
                        out=gu[:], in_=agg1p[u * P : (u + 1) * P, :]
                    )
                else:
                    nc.gpsimd.indirect_dma_start(
                        out=gu[:],
                        out_offset=None,
                        in_=src_tab[:],
                        in_offset=bass.IndirectOffsetOnAxis(
                            ap=idxUall[:, u : u + 1], axis=0
                        ),
                    )
                aT = []
                for fi in range(NFI1):
                    pt = ps.tile([P, P], bf16, tag="ps", name="pt")
                    nc.tensor.transpose(
                        out=pt[:], in_=gu[:, fi * P : (fi + 1) * P], identity=idt[:]
                    )
                    a = ap.tile([P, P], bf16, tag="aT", name="aTt", bufs=8)
                    if fi % 2 == 0:
                        nc.vector.tensor_copy(out=a[:], in_=pt[:])
                    else:
                        nc.scalar.activation(
                            out=a[:], in_=pt[:],
                            func=mybir.ActivationFunctionType.Copy,
                        )
                    aT.append(a)
                return aT

            def rec_back(u, aT):
                """GEMM + relu + writeback for chunk u."""
                h1u_t = hp.tile([P, H], bf16, tag="hout")
                pz = [ps.tile([P, D], f32, tag="ps", name="pz") for _ in range(NFI1)]
                if use_b1:
                    for fo in range(NFI1):
                        nc.tensor.matmul(
                            out=pz[fo][:],
                            lhsT=ones1[:1, :],
                            rhs=b1sb[:1, fo * D : (fo + 1) * D],
                            start=True,
                            stop=False,
                        )
                for fi in range(NFI1):
                    for fo in range(NFI1):
                        nc.tensor.matmul(
                            out=pz[fo][:],
                            lhsT=aT[fi][:],
                            rhs=w1sb[fi][:, fo * D : (fo + 1) * D],
                            start=(fi == 0 and not use_b1),
                            stop=(fi == NFI1 - 1),
                        )
                for fo in range(NFI1):
                    nc.scalar.activation(
                        out=h1u_t[:, fo * D : (fo + 1) * D], in_=pz[fo][:], func=relu
                    )
                nc.sync.dma_start(out=h1u[u * P : (u + 1) * P, :], in_=h1u_t[:])


            # W2/W3 resident loads: issued after L1's input stream so they
            # ride the AllGather shadow instead of delaying the first dtile.
            w2sb = []
            for fi in range(NFI2):
                w = wp.tile([P, H], bf16, tag="w2", name="w2sb", bufs=NFI2)
                nc.sync.dma_start(out=w[:], in_=w2_t[fi * P : (fi + 1) * P, :])
                w2sb.append(w)
            w3sb = []
            for fo in range(NFI2):
                w = wp.tile([P, B], bf16, tag="w3", name="w3sb", bufs=NFI2)
                nc.sync.dma_start(out=w[:], in_=w3i_t[fo * P : (fo + 1) * P, :])
                w3sb.append(w)

            w1sb = []
            for fi in range(NFI1):
                w = wp.tile([P, H], bf16, tag="w1", name="wsb", bufs=NFI1)
                nc.sync.dma_start(out=w[:], in_=w1_t[fi * P : (fi + 1) * P, :])
                w1sb.append(w)

            # software-pipelined: transpose(u) overlaps GEMM(u-1); own-slab
            # chunks (local agg1p) run inside the AllGather shadow, and the
            # first few are issued before the AllGather so the gpsimd queue
            # has work while the collective's input wait resolves.
            prev = None
            for u in range(4):
                aT = rec_front(u, agg1p)
                if prev is not None:
                    rec_back(prev[0], prev[1])
                prev = (u, aT)

            nc.gpsimd.collective_compute(
                "AllGather",
                mybir.AluOpType.bypass,
                replica_groups=rg,
                ins=[agg1p[:]],
                outs=[agg1f[:]],
            )

            for u in range(4, NUCH):
                aT = rec_front(u, agg1p if u < NOWN else agg1f)
                rec_back(prev[0], prev[1])
                prev = (u, aT)
            rec_back(prev[0], prev[1])

            # ---------------- Layer 2 + Q (dtile pairs) --------------------
            NH = NFI2 // 2  # 8 feature chunks per half-row pass
            h1u_half = h1u[:].rearrange("n (h d) -> (n h) d", h=2)
            QS = SLAB // 4

            def q_quarter_ag(j):
                nc.gpsimd.collective_compute(
                    "AllGather",
                    mybir.AluOpType.bypass,
                    replica_groups=rg,
                    ins=[q_slab[j * QS : (j + 1) * QS, :]],
                    outs=[q_full[j * QS * NCORES : (j + 1) * QS * NCORES, :]],
                )

            for tp in range(NDT // 2):
                # issue quarter AllGathers two pairs after their rows complete:
                # the gpsimd queue runs ~a pair ahead of PE, and a collective's
                # SEQ wait stalls every later gather in the queue
                if tp == 5:
                    q_quarter_ag(0)
                if tp == 6:
                    q_quarter_ag(1)
                aggT = [
                    ap.tile([P, 2 * DT], bf16, tag="aggT2", name="aggTt2", bufs=24)
                    for _ in range(NFI2)
                ]
                for t2 in range(2):
                    t = tp * 2 + t2
                    idx_t = mp.tile([P, 2 * ECH], i32, tag="idx")
                    nc.sync.dma_start(out=idx_t[:], in_=idxL2_t[t])
                    s_t = sp.tile([P, ECH * DT], bf16, tag="s")
                    nc.sync.dma_start(out=s_t[:], in_=s_tab[t])

                    for hf in range(2):
                        pa = [ps.tile([P, DT], f32, tag="ps", name="pa2") for _ in range(NH)]
                        for c in range(ECH):
                            g = gp.tile([P, H // 2], bf16, tag="g")
                            nc.gpsimd.indirect_dma_start(
                                out=g[:],
                                out_offset=None,
                                in_=h1u_half,
                                in_offset=bass.IndirectOffsetOnAxis(
                                    ap=idx_t[:, hf * ECH + c : hf * ECH + c + 1], axis=0
                                ),
                            )
                            for j in range(NH):
                                nc.tensor.matmul(
                                    out=pa[j][:],
                                    lhsT=g[:, j * P : (j + 1) * P],
                                    rhs=s_t[:, c * DT : (c + 1) * DT],
                                    start=(c == 0),
                                    stop=(c == ECH - 1),
                                )
                        for j in range(NH):
                            fi = hf * NH + j
                            if j % 2 == 0:
                                nc.vector.tensor_copy(
                                    out=aggT[fi][:, t2 * DT : (t2 + 1) * DT], in_=pa[j][:]
                                )
                            else:
                                nc.scalar.activation(
                                    out=aggT[fi][:, t2 * DT : (t2 + 1) * DT],
                                    in_=pa[j][:],
                                    func=mybir.ActivationFunctionType.Copy,
                                )

                pq = [ps.tile([P, B], f32, tag="ps", name="pq") for _ in range(4)]
                for fo in range(NFI2):
                    pz = ps.tile([P, 2 * DT], f32, tag="ps", name="pz2")
                    for fi in range(NFI2):
                        nc.tensor.matmul(
                            out=pz[:],
                            lhsT=w2sb[fi][:, fo * P : (fo + 1) * P],
                            rhs=aggT[fi][:],
                            start=(fi == 0),
                            stop=(fi == NFI2 - 1),
                        )
                    h2t = ap.tile([P, 2 * DT], bf16, tag="h2", name="h2t", bufs=NFI2)
                    if use_b2:
                        nc.scalar.activation(
                            out=h2t[:], in_=pz[:], func=relu,
                            bias=b2sb[:, fo : fo + 1],
                        )
                    else:
                        nc.scalar.activation(out=h2t[:], in_=pz[:], func=relu)

                    for dh in range(4):
                        nc.tensor.matmul(
                            out=pq[dh][:],
                            lhsT=h2t[:, dh * P : (dh + 1) * P],
                            rhs=w3sb[fo][:],
                            start=(fo == 0),
                            stop=(fo == NFI2 - 1),
                        )
                for dh in range(4):
                    qn = mp.tile([P, B], bf16, tag="qn")
                    nc.vector.tensor_copy(out=qn[:], in_=pq[dh][:])
                    nc.sync.dma_start(
                        out=q_slab[tp * 2 * DT + dh * P : tp * 2 * DT + (dh + 1) * P, :],
                        in_=qn[:],
                    )



            q_quarter_ag(2)
            q_quarter_ag(3)

            # ---------------- Layer 3 (= output) ---------------------------
            # SBUF fp32 accumulators per dtile; single-shot matmul per chunk +
            # DVE add. Chunk order is A-half-gated chunks (all dtiles) first,
            # so their gathers run under the second Q AllGather, then B chunks.
            idxall = cp.tile([P, NDT * ECH], i32, tag="idxall")
            nc.sync.dma_start(out=idxall[:], in_=idxQ2_t[:])
            acc = [
                ap.tile([B, DT], f32, tag="acc", name="acc3", bufs=NDT)
                for _ in range(NDT)
            ]
            first = [True] * NDT
            ordered = sorted(
                ((t, c) for t in range(NDT) for c in range(ECH)),
                key=lambda tc: gateQ[tc[0]][tc[1]],
            )
            for t, c in ordered:
                s3 = mp.tile([P, DT], bf16, tag="s3", bufs=8)
                nc.sync.dma_start(
                    out=s3[:], in_=s_tab[t][:, c * DT : (c + 1) * DT]
                )
                g = gp.tile([P, B], bf16, tag="g3", bufs=8)
                gq = gateQ[t][c]
                src_ap = q_full[0 : (gq + 1) * (SLAB // 4) * NCORES, :]
                nc.gpsimd.indirect_dma_start(
                    out=g[:],
                    out_offset=None,
                    in_=src_ap,
                    in_offset=bass.IndirectOffsetOnAxis(
                        ap=idxall[:, t * ECH + c : t * ECH + c + 1], axis=0
                    ),
                )
                pa = ps.tile([B, DT], f32, tag="ps", name="pa3")
                nc.tensor.matmul(
                    out=pa[:], lhsT=g[:], rhs=s3[:], start=True, stop=True
                )
                if first[t]:
                    nc.vector.tensor_copy(out=acc[t][:], in_=pa[:])
                    first[t] = False
                else:
                    nc.vector.tensor_tensor(
                        out=acc[t][:], in0=acc[t][:], in1=pa[:],
                        op=mybir.AluOpType.add,
                    )
            for t in range(NDT):
                nc.sync.dma_start(out=out_t[:, t * DT : (t + 1) * DT], in_=acc[t][:])

    nc.finalize()
    return nc


_CACHE: dict = {}


def kernel(**inputs: np.ndarray) -> np.ndarray:
    import ml_dtypes

    nodes = np.asarray(inputs["nodes"], dtype=np.float32)
    edge_index = np.asarray(inputs["edge_index"])
    img = np.asarray(inputs["img"], dtype=np.float32)
    W1 = np.asarray(inputs["W1"], dtype=np.float32)
    b1 = np.asarray(inputs["b1"], dtype=np.float32)
    W2 = np.asarray(inputs["W2"], dtype=np.float32)
    b2 = np.asarray(inputs["b2"], dtype=np.float32)
    W3 = np.asarray(inputs["W3"], dtype=np.float32)
    b3 = np.asarray(inputs["b3"], dtype=np.float32)

    ECH, idxA, idxB, S, gateQ = _preprocess(edge_index)
    S = S.astype(ml_dtypes.bfloat16)
    use_b1 = bool(np.any(b1))
    use_b2 = bool(np.any(b2))

    # per-core source rows: all 3584 own-slab rows first (recomputed locally,
    # hidden under the AllGather), then unique remote rows.
    uniq = []   # [NCORES] arrays of remote agg1f row ids, sorted
    idxL2 = []  # [NCORES][NDT, P, 2*ECH] int32 doubled half-row positions
    for k in range(NCORES):
        own_lo, own_hi = k * SLAB, (k + 1) * SLAB
        rem = np.unique(idxB[k])
        rem = rem[(rem < own_lo) | (rem >= own_hi)]
        uniq.append(rem.astype(np.int32))
        pos_map = np.zeros(NCORES * SLAB, dtype=np.int32)
        pos_map[own_lo:own_hi] = np.arange(SLAB)
        pos_map[rem] = SLAB + np.arange(len(rem))
        posk = pos_map[idxB[k]]  # [NDT, P, ECH]
        idxL2.append(np.concatenate([2 * posk, 2 * posk + 1], axis=2))
    NOWN = SLAB // P
    NUCH = NOWN + max(-(-len(u) // P) for u in uniq)
    # q_full is quarter-major: rows [j*7168:(j+1)*7168) hold quarter j
    # (cores' slab rows j*896..j*896+895, core-major within the quarter)
    q_core = idxB // SLAB
    q_r = idxB % SLAB
    QS = SLAB // 4
    idxQ = (
        (q_r // QS) * (NCORES * QS) + q_core * QS + (q_r % QS)
    ).astype(np.int32)

    key = (ECH, NUCH, gateQ, use_b1, use_b2)
    if key not in _CACHE:
        _CACHE[key] = _build(ECH, NUCH, gateQ, use_b1, use_b2)
    nc = _CACHE[key]

    w3img = (W3.astype(np.float32) @ img.astype(np.float32).T).astype(
        ml_dtypes.bfloat16
    )  # [H, B]
    outbias = img @ b3  # [B]

    nodes_r = nodes.astype(ml_dtypes.bfloat16)
    w1_r = W1.astype(ml_dtypes.bfloat16)
    w2_r = W2.astype(ml_dtypes.bfloat16)
    b1_r = b1.reshape(1, H).astype(ml_dtypes.bfloat16)
    b2_r = _round_fp32r(np.ascontiguousarray(b2.reshape(NFI2, P).T))
    ident = np.eye(P, dtype=ml_dtypes.bfloat16)

    in_maps = []
    for k in range(NCORES):
        g1 = nodes_r[idxA[k]].reshape(NDT, P, ECH * D)
        u_pad = np.zeros(NUCH * P, dtype=np.int32)
        u_pad[:SLAB] = np.arange(SLAB)  # own rows: local agg1p row ids
        u_pad[SLAB : SLAB + len(uniq[k])] = uniq[k]
        in_maps.append(
            {
                "G1": np.ascontiguousarray(g1),
                "W1": w1_r,
                "W2": w2_r,
                "W3img": w3img,
                "b1": b1_r,
                "b2": b2_r,
                "IDENT": ident,
                "idxUT": np.ascontiguousarray(u_pad.reshape(NUCH, P).T),
                "idxL2": np.ascontiguousarray(idxL2[k]),
                "idxQ2": np.ascontiguousarray(
                    idxQ[k].transpose(1, 0, 2).reshape(P, NDT * ECH)
                ),
                "S": np.ascontiguousarray(S[k]),
            }
        )

    res = run_bass_kernel_spmd(nc, in_maps, core_ids=list(range(NCORES)))

    full = np.concatenate([res.results[k]["out"] for k in range(NCORES)], axis=1)
    n_ids = np.arange(N_SKIP, N)
    cols = (n_ids // NODES_PER) * SLAB + (n_ids % NODES_PER)
    out = full[:, cols] + outbias[:, None]
    return out.astype(np.float32)


if __name__ == "__main__":
    rng = np.random.default_rng(0)
    ins = {
        "nodes": rng.standard_normal((N, D)).astype(np.float32),
        "edge_index": rng.integers(0, N, size=(2, E)).astype(np.int64),
        "img": rng.standard_normal((B, D)).astype(np.float32),
        "W1": (rng.standard_normal((D, H)) * 0.02).astype(np.float32),
        "b1": np.zeros(H, np.float32),
        "W2": (rng.standard_normal((H, H)) * 0.02).astype(np.float32),
        "b2": np.zeros(H, np.float32),
        "W3": (rng.standard_normal((H, D)) * 0.02).astype(np.float32),
        "b3": np.zeros(D, np.float32),
    }
    out = kernel(**ins)
    print("out", out.shape, out.dtype, np.abs(out).mean())


# revision 26
# speedup vs baseline: 1.0055x; 1.0055x over previous
"""3-layer GCN + img@pair_embed.T for Trainium2, distributed over 8 NeuronCores.

Strategy (destination-sharded graph parallelism, agg1-exchange variant):
  - Each core owns a contiguous slab of destination nodes (3567, padded 3584).
  - Edges (plus self-loops) are bucketed per 256-destination tile and padded to
    128-edge chunks. Host builds per chunk a dense [128 edges x 256 dests]
    one-hot norm matrix S, so segment-sum aggregation becomes TensorE matmuls.
  - Layer-1 source rows are PRE-GATHERED ON HOST (X is a static input), and the
    layer-1 aggregation computes agg1 = A@X directly in node-row orientation
    (lhsT = S chunk), so agg1 [SLAB, 512] is written without any transpose.
  - KEY: the cross-core exchange moves agg1 (512 wide) instead of h1 (2048
    wide): ONE AllGather of [SLAB,512] -> [8*SLAB,512] (29MB out) instead of
    117MB. Each core then recomputes h1 = relu(agg1 @ W1) for only the unique
    source rows its layer-2/3 edges touch (~13k rows): gather agg1 rows,
    PE-transpose them into contraction layout, GEMM against resident W1.
  - Layer 2 gathers 1024-wide half-rows of the local recomputed h1_u in two
    passes (PSUM has only 8 accumulation banks), GEMMs in dtile pairs
    (free dim 512), and folds img into layer 3: W3img = W3@img.T, Q = h2@W3img.
  - Layer 3 aggregates 64-wide Q after a small Q AllGather.
  - Everything exchanged/gathered travels bf16; W1 float32r; W2/W3img bf16;
    PSUM accumulation fp32.
"""

import numpy as np

from concourse import bacc, bass, mybir
from concourse import tile as tile_mod
from concourse.bass_utils import run_bass_kernel_spmd

# Problem shapes (hardcoded per spec nn_GraphModel_26268019982828)
N = 28535
E = 113000
D = 512
H = 2048
B = 64
N_SKIP = 115 + 245  # attrs + objs; pair nodes are N_SKIP..N-1

NCORES = 8
NODES_PER = -(-N // NCORES)  # 3567
P = 128
DT = 256  # destination tile width
NDT = 14  # dest tiles per core
SLAB = NDT * DT  # 3584 padded dests per core
NFI1 = D // P  # 4 feature chunks of layer-1 width
NFI2 = H // P  # 16 feature chunks of hidden width

f32 = mybir.dt.float32
f32r = mybir.dt.float32r
bf16 = mybir.dt.bfloat16
i32 = mybir.dt.int32


def _round_fp32r(x: np.ndarray) -> np.ndarray:
    """Round-to-nearest-even fp32 -> fp32r (11-bit mantissa), numpy."""
    u = np.ascontiguousarray(x, dtype=np.float32).view(np.uint32)
    r = u + (0x7FF + ((u >> 12) & np.uint32(1)))
    r &= np.uint32(0xFFFFF000)
    return r.view(np.float32)


def _preprocess(edge_index: np.ndarray):
    """Sort/bucket edges by destination; build gather indices + S blocks."""
    src = np.concatenate([edge_index[0], np.arange(N, dtype=np.int64)])
    dst = np.concatenate([edge_index[1], np.arange(N, dtype=np.int64)])
    deg = np.bincount(dst, minlength=N).astype(np.float32)  # includes loops
    dinv = (1.0 / np.sqrt(deg)).astype(np.float32)
    norm = (dinv[src] * dinv[dst]).astype(np.float32)

    core = (dst // NODES_PER).astype(np.int64)
    local = (dst - core * NODES_PER).astype(np.int64)
    t_idx = local // DT
    d_local = local % DT
    bucket = core * NDT + t_idx

    # secondary key: source's q-quarter so layer-3 chunks gate on the
    # earliest quarter AllGather that covers all their sources
    quart = (src % NODES_PER) // (SLAB // 4)
    order = np.argsort(bucket * 4 + quart, kind="stable")
    src_s = src[order]
    bucket_s = bucket[order]
    dl_s = d_local[order]
    norm_s = norm[order]

    counts = np.bincount(bucket_s, minlength=NCORES * NDT)
    ECH = int(-(-counts.max() // P))

    idxA = np.zeros((NCORES, NDT, P, ECH), dtype=np.int32)
    idxB = np.zeros((NCORES, NDT, P, ECH), dtype=np.int32)
    S = np.zeros((NCORES, NDT, P, ECH * DT), dtype=np.float32)

    starts = np.zeros(NCORES * NDT + 1, dtype=np.int64)
    np.cumsum(counts, out=starts[1:])
    pos = np.arange(len(bucket_s)) - starts[bucket_s]
    c_idx = pos // P
    e_idx = pos % P

    ci = bucket_s // NDT
    ti = bucket_s % NDT
    srcB = (src_s // NODES_PER) * SLAB + (src_s % NODES_PER)
    idxA[ci, ti, e_idx, c_idx] = src_s.astype(np.int32)
    idxB[ci, ti, e_idx, c_idx] = srcB.astype(np.int32)
    S[ci, ti, e_idx, c_idx * DT + dl_s] = norm_s
    # gate[t][c] = max source-quarter of chunk c across cores (pads -> 0)
    quart_s = quart[order]
    cnt = counts.reshape(NCORES, NDT)
    qmax = np.zeros((NCORES, NDT, ECH), dtype=np.int64)
    for k in range(NCORES):
        for t in range(NDT):
            b = k * NDT + t
            qs = quart_s[starts[b] : starts[b] + cnt[k, t]]
            for c in range(ECH):
                last = min((c + 1) * P, cnt[k, t]) - 1
                qmax[k, t, c] = qs[last] if last >= c * P else 0
    gate = tuple(
        tuple(int(qmax[:, t, c].max()) for c in range(ECH)) for t in range(NDT)
    )
    return ECH, idxA, idxB, S, gate


def _build(ECH: int, NUCH: int, gateQ, use_b1: bool, use_b2: bool):
    nc = bacc.Bacc("TRN2", target_bir_lowering=False, num_devices=NCORES)
    NU = NUCH * P  # padded unique-source rows per core

    g1_t = nc.dram_tensor("G1", [NDT, P, ECH * D], bf16, kind="ExternalInput")
    w1_t = nc.dram_tensor("W1", [D, H], bf16, kind="ExternalInput")
    w2_t = nc.dram_tensor("W2", [H, H], bf16, kind="ExternalInput")
    w3i_t = nc.dram_tensor("W3img", [H, B], bf16, kind="ExternalInput")
    b1_t = nc.dram_tensor("b1", [1, H], bf16, kind="ExternalInput")
    b2_t = nc.dram_tensor("b2", [P, NFI2], f32r, kind="ExternalInput")
    idxU_t = nc.dram_tensor("idxUT", [P, NUCH], i32, kind="ExternalInput")
    idxL2_t = nc.dram_tensor("idxL2", [NDT, P, 2 * ECH], i32, kind="ExternalInput")
    idxQ2_t = nc.dram_tensor("idxQ2", [P, NDT * ECH], i32, kind="ExternalInput")
    s_tab = nc.dram_tensor("S", [NDT, P, ECH * DT], bf16, kind="ExternalInput")
    ident_t = nc.dram_tensor("IDENT", [P, P], bf16, kind="ExternalInput")
    out_t = nc.dram_tensor("out", [B, SLAB], f32, kind="ExternalOutput")

    agg1p = nc.dram_tensor("agg1p", [SLAB, D], bf16)
    agg1f = nc.dram_tensor("agg1f", [SLAB * NCORES, D], bf16, addr_space="Shared")
    h1u = nc.dram_tensor("h1u", [NU, H], bf16)
    q_slab = nc.dram_tensor("q_slab", [SLAB, B], bf16)
    q_full = nc.dram_tensor("q_full", [SLAB * NCORES, B], bf16, addr_space="Shared")

    rg = [list(range(NCORES))]

    with tile_mod.TileContext(nc) as tc:
        with (
            tc.tile_pool(name="w", bufs=1) as wp,
            tc.tile_pool(name="gio", bufs=6) as gp,
            tc.tile_pool(name="g1io", bufs=2) as g1p,
            tc.tile_pool(name="stab", bufs=3) as sp,
            tc.tile_pool(name="agg", bufs=1) as ap,
            tc.tile_pool(name="small", bufs=3) as mp,
            tc.tile_pool(name="hout", bufs=2) as hp,
            tc.tile_pool(name="consts", bufs=1) as cp,
            tc.tile_pool(name="ps", bufs=8, space="PSUM") as ps,
        ):
            # --- resident constants (W1 loads deferred past layer 1) ---
            idxUall = cp.tile([P, NUCH], i32, tag="idxUall")
            nc.sync.dma_start(out=idxUall[:], in_=idxU_t[:])
            idt = cp.tile([P, P], bf16, tag="idt")
            nc.sync.dma_start(out=idt[:], in_=ident_t[:])
            if use_b1:
                b1sb = cp.tile([1, H], bf16, tag="b1")
                nc.sync.dma_start(out=b1sb[:], in_=b1_t[:])
                ones1 = cp.tile([1, P], bf16, tag="ones")
                nc.gpsimd.memset(ones1[:], 1.0)
            if use_b2:
                b2sb = cp.tile([P, NFI2], f32r, tag="b2")
                nc.sync.dma_start(out=b2sb[:], in_=b2_t[:])

            relu = mybir.ActivationFunctionType.Relu

            # ---------------- Layer 1: agg1 = A@X  (node-row orientation) ---
            for tp in range(NDT // 2):
                g1s, sts = [], []
                for t2 in range(2):
                    t = tp * 2 + t2
                    g1 = g1p.tile([P, ECH * D], bf16, tag="g1")
                    nc.sync.dma_start(out=g1[:], in_=g1_t[t])
                    s_t = sp.tile([P, ECH * DT], bf16, tag="s")
                    nc.sync.dma_start(out=s_t[:], in_=s_tab[t])
                    g1s.append(g1)
                    sts.append(s_t)
                pd = [ps.tile([P, D], f32, tag="ps", name="pd") for _ in range(4)]
                for t2 in range(2):
                    for c in range(ECH):
                        for dh in range(2):
                            nc.tensor.matmul(
                                out=pd[t2 * 2 + dh][:],
                                lhsT=sts[t2][:, c * DT + dh * P : c * DT + (dh + 1) * P],
                                rhs=g1s[t2][:, c * D : (c + 1) * D],
                                start=(c == 0),
                                stop=(c == ECH - 1),
                            )
                for t2 in range(2):
                    for dh in range(2):
                        a1 = mp.tile([P, D], bf16, tag="a1")
                        if dh % 2 == 0:
                            nc.vector.tensor_copy(out=a1[:], in_=pd[t2 * 2 + dh][:])
                        else:
                            nc.scalar.activation(
                                out=a1[:], in_=pd[t2 * 2 + dh][:],
                                func=mybir.ActivationFunctionType.Copy,
                            )
                        nc.sync.dma_start(
                            out=agg1p[
                                (tp * 2 + t2) * DT + dh * P : (tp * 2 + t2) * DT + (dh + 1) * P, :
                            ],
                            in_=a1[:],
                        )

            NOWN = SLAB // P  # 28 chunks of own-slab rows, recomputed locally

            def rec_front(u, src_tab):
                """Gather + transpose chunk u; returns aT tiles."""
                gu = gp.tile([P, D], bf16, tag="gu")
                if u < NOWN:
                    nc.sync.dma_start(
                        out=gu[:], in_=agg1p[u * P : (u + 1) * P, :]
                    )
                else:
                    nc.gpsimd.indirect_dma_start(
                        out=gu[:],
                        out_offset=None,
                        in_=src_tab[:],
                        in_offset=bass.IndirectOffsetOnAxis(
                            ap=idxUall[:, u : u + 1], axis=0
                        ),
                    )
                aT = []
                for fi in range(NFI1):
                    pt = ps.tile([P, P], bf16, tag="ps", name="pt")
                    nc.tensor.transpose(
                        out=pt[:], in_=gu[:, fi * P : (fi + 1) * P], identity=idt[:]
                    )
                    a = ap.tile([P, P], bf16, tag="aT", name="aTt", bufs=8)
                    if fi % 2 == 0:
                        nc.vector.tensor_copy(out=a[:], in_=pt[:])
                    else:
                        nc.scalar.activation(
                            out=a[:], in_=pt[:],
                            func=mybir.ActivationFunctionType.Copy,
                        )
                    aT.append(a)
                return aT

            def rec_back(u, aT):
                """GEMM + relu + writeback for chunk u."""
                h1u_t = hp.tile([P, H], bf16, tag="hout")
                pz = [ps.tile([P, D], f32, tag="ps", name="pz") for _ in range(NFI1)]
                if use_b1:
                    for fo in range(NFI1):
                        nc.tensor.matmul(
                            out=pz[fo][:],
                            lhsT=ones1[:1, :],
                            rhs=b1sb[:1, fo * D : (fo + 1) * D],
                            start=True,
                            stop=False,
                        )
                for fi in range(NFI1):
                    for fo in range(NFI1):
                        nc.tensor.matmul(
                            out=pz[fo][:],
                            lhsT=aT[fi][:],
                            rhs=w1sb[fi][:, fo * D : (fo + 1) * D],
                            start=(fi == 0 and not use_b1),
                            stop=(fi == NFI1 - 1),
                        )
                for fo in range(NFI1):
                    nc.scalar.activation(
                        out=h1u_t[:, fo * D : (fo + 1) * D], in_=pz[fo][:], func=relu
                    )
                nc.sync.dma_start(out=h1u[u * P : (u + 1) * P, :], in_=h1u_t[:])


            # W2/W3 resident loads: issued after L1's input stream so they
            # ride the AllGather shadow instead of delaying the first dtile.
            w2sb = []
            for fi in range(NFI2):
                w = wp.tile([P, H], bf16, tag="w2", name="w2sb", bufs=NFI2)
                nc.sync.dma_start(out=w[:], in_=w2_t[fi * P : (fi + 1) * P, :])
                w2sb.append(w)
            w3sb = []
            for fo in range(NFI2):
                w = wp.tile([P, B], bf16, tag="w3", name="w3sb", bufs=NFI2)
                nc.sync.dma_start(out=w[:], in_=w3i_t[fo * P : (fo + 1) * P, :])
                w3sb.append(w)

            w1sb = []
            for fi in range(NFI1):
                w = wp.tile([P, H], bf16, tag="w1", name="wsb", bufs=NFI1)
                nc.sync.dma_start(out=w[:], in_=w1_t[fi * P : (fi + 1) * P, :])
                w1sb.append(w)

            # software-pipelined: transpose(u) overlaps GEMM(u-1); own-slab
            # chunks (local agg1p) run inside the AllGather shadow, and the
            # first few are issued before the AllGather so the gpsimd queue
            # has work while the collective's input wait resolves.
            prev = None
            for u in range(4):
                aT = rec_front(u, agg1p)
                if prev is not None:
                    rec_back(prev[0], prev[1])
                prev = (u, aT)

            nc.gpsimd.collective_compute(
                "AllGather",
                mybir.AluOpType.bypass,
                replica_groups=rg,
                ins=[agg1p[:]],
                outs=[agg1f[:]],
            )

            for u in range(4, NUCH):
                aT = rec_front(u, agg1p if u < NOWN else agg1f)
                rec_back(prev[0], prev[1])
                prev = (u, aT)
            rec_back(prev[0], prev[1])

            # ---------------- Layer 2 + Q (dtile pairs) --------------------
            NH = NFI2 // 2  # 8 feature chunks per half-row pass
            h1u_half = h1u[:].rearrange("n (h d) -> (n h) d", h=2)
            QS = SLAB // 4

            def q_quarter_ag(j):
                nc.gpsimd.collective_compute(
                    "AllGather",
                    mybir.AluOpType.bypass,
                    replica_groups=rg,
                    ins=[q_slab[j * QS : (j + 1) * QS, :]],
                    outs=[q_full[j * QS * NCORES : (j + 1) * QS * NCORES, :]],
                )

            for tp in range(NDT // 2):
                # issue quarter AllGathers two pairs after their rows complete:
                # the gpsimd queue runs ~a pair ahead of PE, and a collective's
                # SEQ wait stalls every later gather in the queue
                for j in range(2):
                    if tp == ((j + 1) * QS - 1) // (2 * DT) + 3:
                        q_quarter_ag(j)
                aggT = [
                    ap.tile([P, 2 * DT], bf16, tag="aggT2", name="aggTt2", bufs=NFI2)
                    for _ in range(NFI2)
                ]
                for t2 in range(2):
                    t = tp * 2 + t2
                    idx_t = mp.tile([P, 2 * ECH], i32, tag="idx")
                    nc.sync.dma_start(out=idx_t[:], in_=idxL2_t[t])
                    s_t = sp.tile([P, ECH * DT], bf16, tag="s")
                    nc.sync.dma_start(out=s_t[:], in_=s_tab[t])

                    for hf in range(2):
                        pa = [ps.tile([P, DT], f32, tag="ps", name="pa2") for _ in range(NH)]
                        for c in range(ECH):
                            g = gp.tile([P, H // 2], bf16, tag="g")
                            nc.gpsimd.indirect_dma_start(
                                out=g[:],
                                out_offset=None,
                                in_=h1u_half,
                                in_offset=bass.IndirectOffsetOnAxis(
                                    ap=idx_t[:, hf * ECH + c : hf * ECH + c + 1], axis=0
                                ),
                            )
                            for j in range(NH):
                                nc.tensor.matmul(
                                    out=pa[j][:],
                                    lhsT=g[:, j * P : (j + 1) * P],
                                    rhs=s_t[:, c * DT : (c + 1) * DT],
                                    start=(c == 0),
                                    stop=(c == ECH - 1),
                                )
                        for j in range(NH):
                            fi = hf * NH + j
                            if j % 2 == 0:
                                nc.vector.tensor_copy(
                                    out=aggT[fi][:, t2 * DT : (t2 + 1) * DT], in_=pa[j][:]
                                )
                            else:
                                nc.scalar.activation(
                                    out=aggT[fi][:, t2 * DT : (t2 + 1) * DT],
                                    in_=pa[j][:],
                                    func=mybir.ActivationFunctionType.Copy,
                                )

                pq = [ps.tile([P, B], f32, tag="ps", name="pq") for _ in range(4)]
                for fo in range(NFI2):
                    pz = ps.tile([P, 2 * DT], f32, tag="ps", name="pz2")
                    for fi in range(NFI2):
                        nc.tensor.matmul(
                            out=pz[:],
                            lhsT=w2sb[fi][:, fo * P : (fo + 1) * P],
                            rhs=aggT[fi][:],
                            start=(fi == 0),
                            stop=(fi == NFI2 - 1),
                        )
                    h2t = ap.tile([P, 2 * DT], bf16, tag="h2", name="h2t", bufs=NFI2)
                    if use_b2:
                        nc.scalar.activation(
                            out=h2t[:], in_=pz[:], func=relu,
                            bias=b2sb[:, fo : fo + 1],
                        )
                    else:
                        nc.scalar.activation(out=h2t[:], in_=pz[:], func=relu)

                    for dh in range(4):
                        nc.tensor.matmul(
                            out=pq[dh][:],
                            lhsT=h2t[:, dh * P : (dh + 1) * P],
                            rhs=w3sb[fo][:],
                            start=(fo == 0),
                            stop=(fo == NFI2 - 1),
                        )
                for dh in range(4):
                    qn = mp.tile([P, B], bf16, tag="qn")
                    nc.vector.tensor_copy(out=qn[:], in_=pq[dh][:])
                    nc.sync.dma_start(
                        out=q_slab[tp * 2 * DT + dh * P : tp * 2 * DT + (dh + 1) * P, :],
                        in_=qn[:],
                    )



            q_quarter_ag(2)
            q_quarter_ag(3)

            # ---------------- Layer 3 (= output) ---------------------------
            # SBUF fp32 accumulators per dtile; single-shot matmul per chunk +
            # DVE add. Chunk order is A-half-gated chunks (all dtiles) first,
            # so their gathers run under the second Q AllGather, then B chunks.
            idxall = cp.tile([P, NDT * ECH], i32, tag="idxall")
            nc.sync.dma_start(out=idxall[:], in_=idxQ2_t[:])
            acc = [
                ap.tile([B, DT], f32, tag="acc", name="acc3", bufs=NDT)
                for _ in range(NDT)
            ]
            first = [True] * NDT
            ordered = sorted(
                ((t, c) for t in range(NDT) for c in range(ECH)),
                key=lambda tc: gateQ[tc[0]][tc[1]],
            )
            for t, c in ordered:
                s3 = mp.tile([P, DT], bf16, tag="s3", bufs=8)
                nc.sync.dma_start(
                    out=s3[:], in_=s_tab[t][:, c * DT : (c + 1) * DT]
                )
                g = gp.tile([P, B], bf16, tag="g3", bufs=12)
                gq = gateQ[t][c]
                src_ap = q_full[0 : (gq + 1) * (SLAB // 4) * NCORES, :]
                nc.gpsimd.indirect_dma_start(
                    out=g[:],
                    out_offset=None,
                    in_=src_ap,
                    in_offset=bass.IndirectOffsetOnAxis(
                        ap=idxall[:, t * ECH + c : t * ECH + c + 1], axis=0
                    ),
                )
                pa = ps.tile([B, DT], f32, tag="ps", name="pa3")
                nc.tensor.matmul(
                    out=pa[:], lhsT=g[:], rhs=s3[:], start=True, stop=True
                )
                if first[t]:
                    nc.vector.tensor_copy(out=acc[t][:], in_=pa[:])
                    first[t] = False
                else:
                    nc.vector.tensor_tensor(
                        out=acc[t][:], in0=acc[t][:], in1=pa[:],
                        op=mybir.AluOpType.add,
                    )
            for t in range(NDT):
                nc.sync.dma_start(out=out_t[:, t * DT : (t + 1) * DT], in_=acc[t][:])

    nc.finalize()
    return nc


_CACHE: dict = {}


def kernel(**inputs: np.ndarray) -> np.ndarray:
    import ml_dtypes

    nodes = np.asarray(inputs["nodes"], dtype=np.float32)
    edge_index = np.asarray(inputs["edge_index"])
    img = np.asarray(inputs["img"], dtype=np.float32)
    W1 = np.asarray(inputs["W1"], dtype=np.float32)
    b1 = np.asarray(inputs["b1"], dtype=np.float32)
    W2 = np.asarray(inputs["W2"], dtype=np.float32)
    b2 = np.asarray(inputs["b2"], dtype=np.float32)
    W3 = np.asarray(inputs["W3"], dtype=np.float32)
    b3 = np.asarray(inputs["b3"], dtype=np.float32)

    ECH, idxA, idxB, S, gateQ = _preprocess(edge_index)
    S = S.astype(ml_dtypes.bfloat16)
    use_b1 = bool(np.any(b1))
    use_b2 = bool(np.any(b2))

    # per-core source rows: all 3584 own-slab rows first (recomputed locally,
    # hidden under the AllGather), then unique remote rows.
    uniq = []   # [NCORES] arrays of remote agg1f row ids, sorted
    idxL2 = []  # [NCORES][NDT, P, 2*ECH] int32 doubled half-row positions
    for k in range(NCORES):
        own_lo, own_hi = k * SLAB, (k + 1) * SLAB
        rem = np.unique(idxB[k])
        rem = rem[(rem < own_lo) | (rem >= own_hi)]
        uniq.append(rem.astype(np.int32))
        pos_map = np.zeros(NCORES * SLAB, dtype=np.int32)
        pos_map[own_lo:own_hi] = np.arange(SLAB)
        pos_map[rem] = SLAB + np.arange(len(rem))
        posk = pos_map[idxB[k]]  # [NDT, P, ECH]
        idxL2.append(np.concatenate([2 * posk, 2 * posk + 1], axis=2))
    NOWN = SLAB // P
    NUCH = NOWN + max(-(-len(u) // P) for u in uniq)
    # q_full is quarter-major: rows [j*7168:(j+1)*7168) hold quarter j
    # (cores' slab rows j*896..j*896+895, core-major within the quarter)
    q_core = idxB // SLAB
    q_r = idxB % SLAB
    QS = SLAB // 4
    idxQ = (
        (q_r // QS) * (NCORES * QS) + q_core * QS + (q_r % QS)
    ).astype(np.int32)

    key = (ECH, NUCH, gateQ, use_b1, use_b2)
    if key not in _CACHE:
        _CACHE[key] = _build(ECH, NUCH, gateQ, use_b1, use_b2)
    nc = _CACHE[key]

    w3img = (W3.astype(np.float32) @ img.astype(np.float32).T).astype(
        ml_dtypes.bfloat16
    )  # [H, B]
    outbias = img @ b3  # [B]

    nodes_r = nodes.astype(ml_dtypes.bfloat16)
    w1_r = W1.astype(ml_dtypes.bfloat16)
    w2_r = W2.astype(ml_dtypes.bfloat16)
    b1_r = b1.reshape(1, H).astype(ml_dtypes.bfloat16)
    b2_r = _round_fp32r(np.ascontiguousarray(b2.reshape(NFI2, P).T))
    ident = np.eye(P, dtype=ml_dtypes.bfloat16)

    in_maps = []
    for k in range(NCORES):
        g1 = nodes_r[idxA[k]].reshape(NDT, P, ECH * D)
        u_pad = np.zeros(NUCH * P, dtype=np.int32)
        u_pad[:SLAB] = np.arange(SLAB)  # own rows: local agg1p row ids
        u_pad[SLAB : SLAB + len(uniq[k])] = uniq[k]
        in_maps.append(
            {
                "G1": np.ascontiguousarray(g1),
                "W1": w1_r,
                "W2": w2_r,
                "W3img": w3img,
                "b1": b1_r,
                "b2": b2_r,
                "IDENT": ident,
                "idxUT": np.ascontiguousarray(u_pad.reshape(NUCH, P).T),
                "idxL2": np.ascontiguousarray(idxL2[k]),
                "idxQ2": np.ascontiguousarray(
                    idxQ[k].transpose(1, 0, 2).reshape(P, NDT * ECH)
                ),
                "S": np.ascontiguousarray(S[k]),
            }
        )

    res = run_bass_kernel_spmd(nc, in_maps, core_ids=list(range(NCORES)))

    full = np.concatenate([res.results[k]["out"] for k in range(NCORES)], axis=1)
    n_ids = np.arange(N_SKIP, N)
    cols = (n_ids // NODES_PER) * SLAB + (n_ids % NODES_PER)
    out = full[:, cols] + outbias[:, None]
    return out.astype(np.float32)


if __name__ == "__main__":
    rng = np.random.default_rng(0)
    ins = {
        "nodes": rng.standard_normal((N, D)).astype(np.float32),
        "edge_index": rng.integers(0, N, size=(2, E)).astype(np.int64),
        "img": rng.standard_normal((B, D)).astype(np.float32),
        "W1": (rng.standard_normal((D, H)) * 0.02).astype(np.float32),
        "b1": np.zeros(H, np.float32),
        "W2": (rng.standard_normal((H, H)) * 0.02).astype(np.float32),
        "b2": np.zeros(H, np.float32),
        "W3": (rng.standard_normal((H, D)) * 0.02).astype(np.float32),
        "b3": np.zeros(D, np.float32),
    }
    out = kernel(**ins)
    print("out", out.shape, out.dtype, np.abs(out).mean())


# revision 28
# speedup vs baseline: 1.0320x; 1.0264x over previous
"""3-layer GCN + img@pair_embed.T for Trainium2, distributed over 8 NeuronCores.

Strategy (destination-sharded graph parallelism, agg1-exchange variant):
  - Each core owns a contiguous slab of destination nodes (3567, padded 3584).
  - Edges (plus self-loops) are bucketed per 256-destination tile and padded to
    128-edge chunks. Host builds per chunk a dense [128 edges x 256 dests]
    one-hot norm matrix S, so segment-sum aggregation becomes TensorE matmuls.
  - Layer-1 source rows are PRE-GATHERED ON HOST (X is a static input), and the
    layer-1 aggregation computes agg1 = A@X directly in node-row orientation
    (lhsT = S chunk), so agg1 [SLAB, 512] is written without any transpose.
  - KEY: the cross-core exchange moves agg1 (512 wide) instead of h1 (2048
    wide): ONE AllGather of [SLAB,512] -> [8*SLAB,512] (29MB out) instead of
    117MB. Each core then recomputes h1 = relu(agg1 @ W1) for only the unique
    source rows its layer-2/3 edges touch (~13k rows): gather agg1 rows,
    PE-transpose them into contraction layout, GEMM against resident W1.
  - Layer 2 gathers 1024-wide half-rows of the local recomputed h1_u in two
    passes (PSUM has only 8 accumulation banks), GEMMs in dtile pairs
    (free dim 512), and folds img into layer 3: W3img = W3@img.T, Q = h2@W3img.
  - Layer 3 aggregates 64-wide Q after a small Q AllGather.
  - Everything exchanged/gathered travels bf16; W1 float32r; W2/W3img bf16;
    PSUM accumulation fp32.
"""

import numpy as np

from concourse import bacc, bass, mybir
from concourse import tile as tile_mod
from concourse.bass_utils import run_bass_kernel_spmd

# Problem shapes (hardcoded per spec nn_GraphModel_26268019982828)
N = 28535
E = 113000
D = 512
H = 2048
B = 64
N_SKIP = 115 + 245  # attrs + objs; pair nodes are N_SKIP..N-1

NCORES = 8
NODES_PER = -(-N // NCORES)  # 3567
P = 128
DT = 256  # destination tile width
NDT = 14  # dest tiles per core
SLAB = NDT * DT  # 3584 padded dests per core
NFI1 = D // P  # 4 feature chunks of layer-1 width
NFI2 = H // P  # 16 feature chunks of hidden width

f32 = mybir.dt.float32
f32r = mybir.dt.float32r
bf16 = mybir.dt.bfloat16
i32 = mybir.dt.int32


def _round_fp32r(x: np.ndarray) -> np.ndarray:
    """Round-to-nearest-even fp32 -> fp32r (11-bit mantissa), numpy."""
    u = np.ascontiguousarray(x, dtype=np.float32).view(np.uint32)
    r = u + (0x7FF + ((u >> 12) & np.uint32(1)))
    r &= np.uint32(0xFFFFF000)
    return r.view(np.float32)


def _preprocess(edge_index: np.ndarray):
    """Sort/bucket edges by destination; build gather indices + S blocks."""
    src = np.concatenate([edge_index[0], np.arange(N, dtype=np.int64)])
    dst = np.concatenate([edge_index[1], np.arange(N, dtype=np.int64)])
    deg = np.bincount(dst, minlength=N).astype(np.float32)  # includes loops
    dinv = (1.0 / np.sqrt(deg)).astype(np.float32)
    norm = (dinv[src] * dinv[dst]).astype(np.float32)

    core = (dst // NODES_PER).astype(np.int64)
    local = (dst - core * NODES_PER).astype(np.int64)
    t_idx = local // DT
    d_local = local % DT
    bucket = core * NDT + t_idx

    # secondary key: source's q-quarter so layer-3 chunks gate on the
    # earliest quarter AllGather that covers all their sources
    quart = (src % NODES_PER) // (SLAB // 4)
    order = np.argsort(bucket * 4 + quart, kind="stable")
    src_s = src[order]
    bucket_s = bucket[order]
    dl_s = d_local[order]
    norm_s = norm[order]

    counts = np.bincount(bucket_s, minlength=NCORES * NDT)
    ECH = int(-(-counts.max() // P))

    idxA = np.zeros((NCORES, NDT, P, ECH), dtype=np.int32)
    idxB = np.zeros((NCORES, NDT, P, ECH), dtype=np.int32)
    S = np.zeros((NCORES, NDT, P, ECH * DT), dtype=np.float32)

    starts = np.zeros(NCORES * NDT + 1, dtype=np.int64)
    np.cumsum(counts, out=starts[1:])
    pos = np.arange(len(bucket_s)) - starts[bucket_s]
    c_idx = pos // P
    e_idx = pos % P

    ci = bucket_s // NDT
    ti = bucket_s % NDT
    srcB = (src_s // NODES_PER) * SLAB + (src_s % NODES_PER)
    idxA[ci, ti, e_idx, c_idx] = src_s.astype(np.int32)
    idxB[ci, ti, e_idx, c_idx] = srcB.astype(np.int32)
    S[ci, ti, e_idx, c_idx * DT + dl_s] = norm_s
    # gate[t][c] = max source-quarter of chunk c across cores (pads -> 0)
    quart_s = quart[order]
    cnt = counts.reshape(NCORES, NDT)
    qmax = np.zeros((NCORES, NDT, ECH), dtype=np.int64)
    for k in range(NCORES):
        for t in range(NDT):
            b = k * NDT + t
            qs = quart_s[starts[b] : starts[b] + cnt[k, t]]
            for c in range(ECH):
                last = min((c + 1) * P, cnt[k, t]) - 1
                qmax[k, t, c] = qs[last] if last >= c * P else 0
    gate = tuple(
        tuple(int(qmax[:, t, c].max()) for c in range(ECH)) for t in range(NDT)
    )
    return ECH, idxA, idxB, S, gate


def _build(ECH: int, NUCH: int, gateQ, use_b1: bool, use_b2: bool):
    nc = bacc.Bacc("TRN2", target_bir_lowering=False, num_devices=NCORES)
    NU = NUCH * P  # padded unique-source rows per core

    g1_t = nc.dram_tensor("G1", [NDT, P, ECH * D], bf16, kind="ExternalInput")
    w1_t = nc.dram_tensor("W1", [D, H], bf16, kind="ExternalInput")
    w2_t = nc.dram_tensor("W2", [H, H], bf16, kind="ExternalInput")
    w3i_t = nc.dram_tensor("W3img", [H, B], bf16, kind="ExternalInput")
    b1_t = nc.dram_tensor("b1", [1, H], bf16, kind="ExternalInput")
    b2_t = nc.dram_tensor("b2", [P, NFI2], f32r, kind="ExternalInput")
    idxU_t = nc.dram_tensor("idxUT", [P, NUCH], i32, kind="ExternalInput")
    idxL2_t = nc.dram_tensor("idxL2", [NDT, P, 2 * ECH], i32, kind="ExternalInput")
    idxQ2_t = nc.dram_tensor("idxQ2", [P, NDT * ECH], i32, kind="ExternalInput")
    s_tab = nc.dram_tensor("S", [NDT, P, ECH * DT], bf16, kind="ExternalInput")
    ident_t = nc.dram_tensor("IDENT", [P, P], bf16, kind="ExternalInput")
    out_t = nc.dram_tensor("out", [B, SLAB], f32, kind="ExternalOutput")

    agg1p = nc.dram_tensor("agg1p", [SLAB, D], bf16)
    agg1f = nc.dram_tensor("agg1f", [SLAB * NCORES, D], bf16, addr_space="Shared")
    h1u = nc.dram_tensor("h1u", [NU, H], bf16)
    q_slab = nc.dram_tensor("q_slab", [SLAB, B], bf16)
    q_full = nc.dram_tensor("q_full", [SLAB * NCORES, B], bf16, addr_space="Shared")

    rg = [list(range(NCORES))]

    with tile_mod.TileContext(nc) as tc:
        with (
            tc.tile_pool(name="w", bufs=1) as wp,
            tc.tile_pool(name="gio", bufs=6) as gp,
            tc.tile_pool(name="g1io", bufs=2) as g1p,
            tc.tile_pool(name="stab", bufs=3) as sp,
            tc.tile_pool(name="agg", bufs=1) as ap,
            tc.tile_pool(name="small", bufs=3) as mp,
            tc.tile_pool(name="hout", bufs=2) as hp,
            tc.tile_pool(name="consts", bufs=1) as cp,
            tc.tile_pool(name="ps", bufs=8, space="PSUM") as ps,
        ):
            # --- resident constants (W1 loads deferred past layer 1) ---
            idxUall = cp.tile([P, NUCH], i32, tag="idxUall")
            nc.sync.dma_start(out=idxUall[:], in_=idxU_t[:])
            idt = cp.tile([P, P], bf16, tag="idt")
            nc.sync.dma_start(out=idt[:], in_=ident_t[:])
            if use_b1:
                b1sb = cp.tile([1, H], bf16, tag="b1")
                nc.sync.dma_start(out=b1sb[:], in_=b1_t[:])
                ones1 = cp.tile([1, P], bf16, tag="ones")
                nc.gpsimd.memset(ones1[:], 1.0)
            if use_b2:
                b2sb = cp.tile([P, NFI2], f32r, tag="b2")
                nc.sync.dma_start(out=b2sb[:], in_=b2_t[:])

            relu = mybir.ActivationFunctionType.Relu

            # ---------------- Layer 1: agg1 = A@X  (node-row orientation) ---
            for tp in range(NDT // 2):
                g1s, sts = [], []
                for t2 in range(2):
                    t = tp * 2 + t2
                    g1 = g1p.tile([P, ECH * D], bf16, tag="g1")
                    nc.sync.dma_start(out=g1[:], in_=g1_t[t])
                    s_t = sp.tile([P, ECH * DT], bf16, tag="s")
                    nc.sync.dma_start(out=s_t[:], in_=s_tab[t])
                    g1s.append(g1)
                    sts.append(s_t)
                pd = [ps.tile([P, D], f32, tag="ps", name="pd") for _ in range(4)]
                for t2 in range(2):
                    for c in range(ECH):
                        for dh in range(2):
                            nc.tensor.matmul(
                                out=pd[t2 * 2 + dh][:],
                                lhsT=sts[t2][:, c * DT + dh * P : c * DT + (dh + 1) * P],
                                rhs=g1s[t2][:, c * D : (c + 1) * D],
                                start=(c == 0),
                                stop=(c == ECH - 1),
                            )
                for t2 in range(2):
                    for dh in range(2):
                        a1 = mp.tile([P, D], bf16, tag="a1")
                        if dh % 2 == 0:
                            nc.vector.tensor_copy(out=a1[:], in_=pd[t2 * 2 + dh][:])
                        else:
                            nc.scalar.activation(
                                out=a1[:], in_=pd[t2 * 2 + dh][:],
                                func=mybir.ActivationFunctionType.Copy,
                            )
                        nc.sync.dma_start(
                            out=agg1p[
                                (tp * 2 + t2) * DT + dh * P : (tp * 2 + t2) * DT + (dh + 1) * P, :
                            ],
                            in_=a1[:],
                        )

            NOWN = SLAB // P  # 28 chunks of own-slab rows, recomputed locally

            def rec_front(u, src_tab):
                """Gather + transpose chunk u; returns aT tiles."""
                gu = gp.tile([P, D], bf16, tag="gu")
                if u < NOWN:
                    nc.sync.dma_start(
                        out=gu[:], in_=agg1p[u * P : (u + 1) * P, :]
                    )
                else:
                    nc.gpsimd.indirect_dma_start(
                        out=gu[:],
                        out_offset=None,
                        in_=src_tab[:],
                        in_offset=bass.IndirectOffsetOnAxis(
                            ap=idxUall[:, u : u + 1], axis=0
                        ),
                    )
                aT = []
                for fi in range(NFI1):
                    pt = ps.tile([P, P], bf16, tag="ps", name="pt")
                    nc.tensor.transpose(
                        out=pt[:], in_=gu[:, fi * P : (fi + 1) * P], identity=idt[:]
                    )
                    a = ap.tile([P, P], bf16, tag="aT", name="aTt", bufs=8)
                    if fi % 2 == 0:
                        nc.vector.tensor_copy(out=a[:], in_=pt[:])
                    else:
                        nc.scalar.activation(
                            out=a[:], in_=pt[:],
                            func=mybir.ActivationFunctionType.Copy,
                        )
                    aT.append(a)
                return aT

            def rec_back(u, aT):
                """GEMM + relu + writeback for chunk u."""
                h1u_t = hp.tile([P, H], bf16, tag="hout")
                pz = [ps.tile([P, D], f32, tag="ps", name="pz") for _ in range(NFI1)]
                if use_b1:
                    for fo in range(NFI1):
                        nc.tensor.matmul(
                            out=pz[fo][:],
                            lhsT=ones1[:1, :],
                            rhs=b1sb[:1, fo * D : (fo + 1) * D],
                            start=True,
                            stop=False,
                        )
                for fi in range(NFI1):
                    for fo in range(NFI1):
                        nc.tensor.matmul(
                            out=pz[fo][:],
                            lhsT=aT[fi][:],
                            rhs=w1sb[fi][:, fo * D : (fo + 1) * D],
                            start=(fi == 0 and not use_b1),
                            stop=(fi == NFI1 - 1),
                        )
                for fo in range(NFI1):
                    nc.scalar.activation(
                        out=h1u_t[:, fo * D : (fo + 1) * D], in_=pz[fo][:], func=relu
                    )
                nc.sync.dma_start(out=h1u[u * P : (u + 1) * P, :], in_=h1u_t[:])


            # W2/W3 resident loads: issued after L1's input stream so they
            # ride the AllGather shadow instead of delaying the first dtile.
            w2sb = []
            for fi in range(NFI2):
                w = wp.tile([P, H], bf16, tag="w2", name="w2sb", bufs=NFI2)
                nc.sync.dma_start(out=w[:], in_=w2_t[fi * P : (fi + 1) * P, :])
                w2sb.append(w)
            w3sb = []
            for fo in range(NFI2):
                w = wp.tile([P, B], bf16, tag="w3", name="w3sb", bufs=NFI2)
                nc.sync.dma_start(out=w[:], in_=w3i_t[fo * P : (fo + 1) * P, :])
                w3sb.append(w)

            w1sb = []
            for fi in range(NFI1):
                w = wp.tile([P, H], bf16, tag="w1", name="wsb", bufs=NFI1)
                nc.sync.dma_start(out=w[:], in_=w1_t[fi * P : (fi + 1) * P, :])
                w1sb.append(w)

            # software-pipelined: transpose(u) overlaps GEMM(u-1); own-slab
            # chunks (local agg1p) run inside the AllGather shadow, and the
            # first few are issued before the AllGather so the gpsimd queue
            # has work while the collective's input wait resolves.
            prev = None
            for u in range(4):
                aT = rec_front(u, agg1p)
                if prev is not None:
                    rec_back(prev[0], prev[1])
                prev = (u, aT)

            nc.gpsimd.collective_compute(
                "AllGather",
                mybir.AluOpType.bypass,
                replica_groups=rg,
                ins=[agg1p[:]],
                outs=[agg1f[:]],
            )

            for u in range(4, NUCH):
                aT = rec_front(u, agg1p if u < NOWN else agg1f)
                rec_back(prev[0], prev[1])
                prev = (u, aT)
            rec_back(prev[0], prev[1])

            # ---------------- Layer 2 + Q (dtile pairs) --------------------
            NH = NFI2 // 2  # 8 feature chunks per half-row pass
            h1u_half = h1u[:].rearrange("n (h d) -> (n h) d", h=2)
            QS = SLAB // 4

            def q_quarter_ag(j):
                nc.gpsimd.collective_compute(
                    "AllGather",
                    mybir.AluOpType.bypass,
                    replica_groups=rg,
                    ins=[q_slab[j * QS : (j + 1) * QS, :]],
                    outs=[q_full[j * QS * NCORES : (j + 1) * QS * NCORES, :]],
                )

            for tp in range(NDT // 2):
                # issue quarter AllGathers two pairs after their rows complete:
                # the gpsimd queue runs ~a pair ahead of PE, and a collective's
                # SEQ wait stalls every later gather in the queue
                for j in range(2):
                    if tp == ((j + 1) * QS - 1) // (2 * DT) + 3:
                        q_quarter_ag(j)
                aggT = [
                    ap.tile([P, 2 * DT], bf16, tag="aggT2", name="aggTt2", bufs=NFI2)
                    for _ in range(NFI2)
                ]
                for t2 in range(2):
                    t = tp * 2 + t2
                    idx_t = mp.tile([P, 2 * ECH], i32, tag="idx")
                    nc.sync.dma_start(out=idx_t[:], in_=idxL2_t[t])
                    s_t = sp.tile([P, ECH * DT], bf16, tag="s")
                    nc.sync.dma_start(out=s_t[:], in_=s_tab[t])

                    for hf in range(2):
                        pa = [ps.tile([P, DT], f32, tag="ps", name="pa2") for _ in range(NH)]
                        for c in range(ECH):
                            g = gp.tile([P, H // 2], bf16, tag="g", bufs=8)
                            nc.gpsimd.indirect_dma_start(
                                out=g[:],
                                out_offset=None,
                                in_=h1u_half,
                                in_offset=bass.IndirectOffsetOnAxis(
                                    ap=idx_t[:, hf * ECH + c : hf * ECH + c + 1], axis=0
                                ),
                            )
                            for j in range(NH):
                                nc.tensor.matmul(
                                    out=pa[j][:],
                                    lhsT=g[:, j * P : (j + 1) * P],
                                    rhs=s_t[:, c * DT : (c + 1) * DT],
                                    start=(c == 0),
                                    stop=(c == ECH - 1),
                                )
                        for j in range(NH):
                            fi = hf * NH + j
                            if j % 2 == 0:
                                nc.vector.tensor_copy(
                                    out=aggT[fi][:, t2 * DT : (t2 + 1) * DT], in_=pa[j][:]
                                )
                            else:
                                nc.scalar.activation(
                                    out=aggT[fi][:, t2 * DT : (t2 + 1) * DT],
                                    in_=pa[j][:],
                                    func=mybir.ActivationFunctionType.Copy,
                                )

                pq = [ps.tile([P, B], f32, tag="ps", name="pq") for _ in range(4)]
                for fo in range(NFI2):
                    pz = ps.tile([P, 2 * DT], f32, tag="ps", name="pz2")
                    for fi in range(NFI2):
                        nc.tensor.matmul(
                            out=pz[:],
                            lhsT=w2sb[fi][:, fo * P : (fo + 1) * P],
                            rhs=aggT[fi][:],
                            start=(fi == 0),
                            stop=(fi == NFI2 - 1),
                        )
                    h2t = ap.tile([P, 2 * DT], bf16, tag="h2", name="h2t", bufs=NFI2)
                    if use_b2:
                        nc.scalar.activation(
                            out=h2t[:], in_=pz[:], func=relu,
                            bias=b2sb[:, fo : fo + 1],
                        )
                    else:
                        nc.scalar.activation(out=h2t[:], in_=pz[:], func=relu)

                    for dh in range(4):
                        nc.tensor.matmul(
                            out=pq[dh][:],
                            lhsT=h2t[:, dh * P : (dh + 1) * P],
                            rhs=w3sb[fo][:],
                            start=(fo == 0),
                            stop=(fo == NFI2 - 1),
                        )
                for dh in range(4):
                    qn = mp.tile([P, B], bf16, tag="qn")
                    nc.vector.tensor_copy(out=qn[:], in_=pq[dh][:])
                    nc.sync.dma_start(
                        out=q_slab[tp * 2 * DT + dh * P : tp * 2 * DT + (dh + 1) * P, :],
                        in_=qn[:],
                    )



            q_quarter_ag(2)
            q_quarter_ag(3)

            # ---------------- Layer 3 (= output) ---------------------------
            # SBUF fp32 accumulators per dtile; single-shot matmul per chunk +
            # DVE add. Chunk order is A-half-gated chunks (all dtiles) first,
            # so their gathers run under the second Q AllGather, then B chunks.
            idxall = cp.tile([P, NDT * ECH], i32, tag="idxall")
            nc.sync.dma_start(out=idxall[:], in_=idxQ2_t[:])
            acc = [
                ap.tile([B, DT], f32, tag="acc", name="acc3", bufs=NDT)
                for _ in range(NDT)
            ]
            first = [True] * NDT
            ordered = sorted(
                ((t, c) for t in range(NDT) for c in range(ECH)),
                key=lambda tc: gateQ[tc[0]][tc[1]],
            )
            for t, c in ordered:
                s3 = mp.tile([P, DT], bf16, tag="s3", bufs=6)
                nc.sync.dma_start(
                    out=s3[:], in_=s_tab[t][:, c * DT : (c + 1) * DT]
                )
                g = gp.tile([P, B], bf16, tag="g3", bufs=10)
                gq = gateQ[t][c]
                src_ap = q_full[0 : (gq + 1) * (SLAB // 4) * NCORES, :]
                nc.gpsimd.indirect_dma_start(
                    out=g[:],
                    out_offset=None,
                    in_=src_ap,
                    in_offset=bass.IndirectOffsetOnAxis(
                        ap=idxall[:, t * ECH + c : t * ECH + c + 1], axis=0
                    ),
                )
                pa = ps.tile([B, DT], f32, tag="ps", name="pa3")
                nc.tensor.matmul(
                    out=pa[:], lhsT=g[:], rhs=s3[:], start=True, stop=True
                )
                if first[t]:
                    nc.vector.tensor_copy(out=acc[t][:], in_=pa[:])
                    first[t] = False
                else:
                    nc.vector.tensor_tensor(
                        out=acc[t][:], in0=acc[t][:], in1=pa[:],
                        op=mybir.AluOpType.add,
                    )
            for t in range(NDT):
                nc.sync.dma_start(out=out_t[:, t * DT : (t + 1) * DT], in_=acc[t][:])

    nc.finalize()
    return nc


_CACHE: dict = {}


def kernel(**inputs: np.ndarray) -> np.ndarray:
    import ml_dtypes

    nodes = np.asarray(inputs["nodes"], dtype=np.float32)
    edge_index = np.asarray(inputs["edge_index"])
    img = np.asarray(inputs["img"], dtype=np.float32)
    W1 = np.asarray(inputs["W1"], dtype=np.float32)
    b1 = np.asarray(inputs["b1"], dtype=np.float32)
    W2 = np.asarray(inputs["W2"], dtype=np.float32)
    b2 = np.asarray(inputs["b2"], dtype=np.float32)
    W3 = np.asarray(inputs["W3"], dtype=np.float32)
    b3 = np.asarray(inputs["b3"], dtype=np.float32)

    ECH, idxA, idxB, S, gateQ = _preprocess(edge_index)
    S = S.astype(ml_dtypes.bfloat16)
    use_b1 = bool(np.any(b1))
    use_b2 = bool(np.any(b2))

    # per-core source rows: all 3584 own-slab rows first (recomputed locally,
    # hidden under the AllGather), then unique remote rows.
    uniq = []   # [NCORES] arrays of remote agg1f row ids, sorted
    idxL2 = []  # [NCORES][NDT, P, 2*ECH] int32 doubled half-row positions
    for k in range(NCORES):
        own_lo, own_hi = k * SLAB, (k + 1) * SLAB
        rem = np.unique(idxB[k])
        rem = rem[(rem < own_lo) | (rem >= own_hi)]
        uniq.append(rem.astype(np.int32))
        pos_map = np.zeros(NCORES * SLAB, dtype=np.int32)
        pos_map[own_lo:own_hi] = np.arange(SLAB)
        pos_map[rem] = SLAB + np.arange(len(rem))
        posk = pos_map[idxB[k]]  # [NDT, P, ECH]
        idxL2.append(np.concatenate([2 * posk, 2 * posk + 1], axis=2))
    NOWN = SLAB // P
    NUCH = NOWN + max(-(-len(u) // P) for u in uniq)
    # q_full is quarter-major: rows [j*7168:(j+1)*7168) hold quarter j
    # (cores' slab rows j*896..j*896+895, core-major within the quarter)
    q_core = idxB // SLAB
    q_r = idxB % SLAB
    QS = SLAB // 4
    idxQ = (
        (q_r // QS) * (NCORES * QS) + q_core * QS + (q_r % QS)
    ).astype(np.int32)

    key = (ECH, NUCH, gateQ, use_b1, use_b2)
    if key not in _CACHE:
        _CACHE[key] = _build(ECH, NUCH, gateQ, use_b1, use_b2)
    nc = _CACHE[key]

    w3img = (W3.astype(np.float32) @ img.astype(np.float32).T).astype(
        ml_dtypes.bfloat16
    )  # [H, B]
    outbias = img @ b3  # [B]

    nodes_r = nodes.astype(ml_dtypes.bfloat16)
    w1_r = W1.astype(ml_dtypes.bfloat16)
    w2_r = W2.astype(ml_dtypes.bfloat16)
    b1_r = b1.reshape(1, H).astype(ml_dtypes.bfloat16)
    b2_r = _round_fp32r(np.ascontiguousarray(b2.reshape(NFI2, P).T))
    ident = np.eye(P, dtype=ml_dtypes.bfloat16)

    in_maps = []
    for k in range(NCORES):
        g1 = nodes_r[idxA[k]].reshape(NDT, P, ECH * D)
        u_pad = np.zeros(NUCH * P, dtype=np.int32)
        u_pad[:SLAB] = np.arange(SLAB)  # own rows: local agg1p row ids
        u_pad[SLAB : SLAB + len(uniq[k])] = uniq[k]
        in_maps.append(
            {
                "G1": np.ascontiguousarray(g1),
                "W1": w1_r,
                "W2": w2_r,
                "W3img": w3img,
                "b1": b1_r,
                "b2": b2_r,
                "IDENT": ident,
                "idxUT": np.ascontiguousarray(u_pad.reshape(NUCH, P).T),
                "idxL2": np.ascontiguousarray(idxL2[k]),
                "idxQ2": np.ascontiguousarray(
                    idxQ[k].transpose(1, 0, 2).reshape(P, NDT * ECH)
                ),
                "S": np.ascontiguousarray(S[k]),
            }
        )

    res = run_bass_kernel_spmd(nc, in_maps, core_ids=list(range(NCORES)))

    full = np.concatenate([res.results[k]["out"] for k in range(NCORES)], axis=1)
    n_ids = np.arange(N_SKIP, N)
    cols = (n_ids // NODES_PER) * SLAB + (n_ids % NODES_PER)
    out = full[:, cols] + outbias[:, None]
    return out.astype(np.float32)


if __name__ == "__main__":
    rng = np.random.default_rng(0)
    ins = {
        "nodes": rng.standard_normal((N, D)).astype(np.float32),
        "edge_index": rng.integers(0, N, size=(2, E)).astype(np.int64),
        "img": rng.standard_normal((B, D)).astype(np.float32),
        "W1": (rng.standard_normal((D, H)) * 0.02).astype(np.float32),
        "b1": np.zeros(H, np.float32),
        "W2": (rng.standard_normal((H, H)) * 0.02).astype(np.float32),
        "b2": np.zeros(H, np.float32),
        "W3": (rng.standard_normal((H, D)) * 0.02).astype(np.float32),
        "b3": np.zeros(D, np.float32),
    }
    out = kernel(**ins)
    print("out", out.shape, out.dtype, np.abs(out).mean())
